# revision 52
# baseline (speedup 1.0000x reference)
"""Multi-head attention (B=2, S=2048, D=1024, H=16) on 8 Trainium2 cores.

Sharding: tensor-parallel over heads (4 groups of 4 heads) x data-parallel
over batch (2). Core c handles batch c//4, head group c%4. The output
projection computes partials for all 1024 out-cols from the local 256 ctx
dims, then ReduceScatter(add) over the 4-core group.

Optimized (465us baseline -> ~256us HW exec), all-f16 compute with fp32
PSUM accumulate (a TRN2 requirement):
  - x made fully SBUF-resident so mid-kernel DMA never competes with the
    ReduceScatter data phases
  - scores for both packed heads land in one [128,2,512] PSUM tile, one
    bank per head; ONE exp ACTIVATE covers both (halves ScalarE
    per-instruction overhead); causal mask as post-exp f16 tril multiply
  - softmax denominators (row 64 of ctx' via ones column in V') are
    transposed onto 128 partitions with tiny PE matmuls, reciprocal'd
    there (~150ns vs 3.3us on one DVE lane), and broadcast back via
    selector-outer-product matmuls
  - emission-interleave work queue: proj(sg+1)/outproj(sg-1) PE tasks are
    drained between attention i-blocks so the PE FIFO never idles
  - RS staging on the gpsimd queue, unique DRAM tile tags per window, and
    rsout->out DMAs deferred to the end so the collectives pipeline
"""
import os
import numpy as np

import concourse.bass as bass
import concourse.mybir as mybir
import concourse.tile as tile
import bass_rust as _bass_rust
from concourse.bass_utils import run_bass_kernel_spmd

dt = mybir.dt
AF = mybir.ActivationFunctionType
ALU = mybir.AluOpType

B, S, D, H = 2, 2048, 1024, 16
DK = D // H          # 64
HL = 4               # heads per core
DL = HL * DK         # 256 local head dims
NCORE = 8
GROUPS = [[0, 1, 2, 3], [4, 5, 6, 7]]
SQG = 512            # sq group width (one PSUM bank of fp32)
NSQG = S // SQG      # 4
NSK = S // 128       # 16 sk blocks
KCH = D // 128       # 8 contraction chunks for projections
SCALE = 1.0 / float(np.sqrt(np.float32(DK)))

DTNAME = os.environ.get("KERNEL_DT", "f16")
_DT_NP = {"f16": np.float16, "bf16": np.float32, "f32r": np.float32}
_DT_MY = {"f16": dt.float16, "bf16": dt.bfloat16, "f32r": dt.float32r}

LAST_RESULT = None   # BassKernelResults of the most recent run (profiling)
_CACHE = {}          # (dtname, causal) -> built Bass


def _split_multiwait(nc):
    """The walrus supports one sync-wait per instruction; Tile emits several.
    Hoist all but the last wait of each instruction onto single-wait NOPs
    placed immediately before it on the same engine."""
    for bbw in nc.bb_map.values():
        insts = bbw.bb.instructions
        out = []
        for inst in insts:
            si = inst.sync_info
            waits = list(si.on_wait or []) if si is not None else []
            if len(waits) > 1:
                for w in waits[:-1]:
                    nop = _bass_rust.InstNoOp(
                        name=nc.get_next_instruction_name(), ins=[], outs=[])
                    nop.engine = inst.engine
                    nop.bass_nofuse = True
                    nop.sync_info = mybir.SyncInfo(on_wait=[w], on_update=[])
                    nc.register_instruction(nop)
                    out.append(nop)
                inst.sync_info = mybir.SyncInfo(
                    on_wait=[waits[-1]], on_update=list(si.on_update or []))
            out.append(inst)
        insts[:] = out


def _build(dtname: str, causal: bool):
    DT = _DT_MY[dtname]
    nc = bass.Bass(num_devices=NCORE)

    xq = nc.declare_dram_parameter("xq", [D, S], DT, isOutput=False)
    xk = nc.declare_dram_parameter("xk", [D, S], DT, isOutput=False)
    xv = nc.declare_dram_parameter("xv", [D, S], DT, isOutput=False)
    wq = nc.declare_dram_parameter("wq", [D, DL], DT, isOutput=False)
    wk = nc.declare_dram_parameter("wk", [D, DL], DT, isOutput=False)
    wv = nc.declare_dram_parameter("wv", [D, DL], DT, isOutput=False)
    # wo holds this core's ROW quarter [DL, D] (partials + ReduceScatter)
    wo = nc.declare_dram_parameter("wo", [DL, D], DT, isOutput=False)
    # wo2 holds the COLUMN quarter [D, DL]: the last window's out-proj runs
    # locally after an AllGather of ctx (cheaper tail than a ReduceScatter)
    wo2 = nc.declare_dram_parameter("wo2", [D, DL], DT, isOutput=False)
    # multiplicative tril tile in scores_T layout: (sk p, sq f) valid iff p<=f
    mask_t = nc.declare_dram_parameter("mask_t", [128, 128], DT, isOutput=False)
    ones_c = nc.declare_dram_parameter("ones_c", [128, 64], DT, isOutput=False)
    ones_r = nc.declare_dram_parameter("ones_r", [1, 64], DT, isOutput=False)
    idn128_p = nc.declare_dram_parameter("idn128", [128, 128], DT,
                                         isOutput=False)
    sel_p = nc.declare_dram_parameter("sel", [8, 8, 64], DT, isOutput=False)
    out = nc.declare_dram_parameter("out", [2, 128, S], dt.float16, isOutput=True)

    with tile.TileContext(nc) as tc:
        with (
            tc.tile_pool(name="wpool", bufs=1) as wpool,
            tc.tile_pool(name="apool", bufs=1) as apool,
            tc.tile_pool(name="epool", bufs=4) as epool,
            tc.tile_pool(name="opool", bufs=2) as opool,
            tc.tile_pool(name="psS", bufs=2, space="PSUM") as psS,
            tc.tile_pool(name="psC", bufs=1, space="PSUM") as psC,
            tc.tile_pool(name="psP", bufs=2, space="PSUM") as psP,
            tc.tile_pool(name="dram", bufs=1, space="DRAM") as drp,
        ):
            # ---- resident weights / activations / constants ----
            # ALL of x is made SBUF-resident (6 MB f16): mid-kernel DMA
            # traffic then never competes with the ReduceScatter data phases
            # (which starve concurrent HBM DMA and stalled the PE for ~15us
            # per RS in earlier versions).
            wq_sb = wpool.tile([128, KCH, DL], DT, tag="wq")
            wk_sb = wpool.tile([128, KCH, DL], DT, tag="wk")
            wv_sb = wpool.tile([128, KCH, DL], DT, tag="wv")
            wo_sb = wpool.tile([128, 2, D], DT, tag="wo")
            xq_sb = wpool.tile([128, KCH, S], DT, tag="xq")
            xk_sb = wpool.tile([128, KCH, S], DT, tag="xk")
            xv_sb = wpool.tile([128, KCH, S], DT, tag="xv")
            # first-use order: weight chunk k with x chunk (k, sg=0) pairs,
            # then the later sq groups' x slices stream in behind them.
            # The k-tensor traffic is issued on the ScalarE HWDGE ring
            # (idle during startup) so the two descriptor queues drain the
            # ~5 MB first-window working set in parallel; wo is deferred
            # behind the sg=1 slices (first needed at outproj(0), ~40us in).
            for w_in, w_sb, x_in, x_sb, q in (
                    (wq, wq_sb, xq, xq_sb, nc.sync),
                    (wk, wk_sb, xk, xk_sb, nc.scalar)):
                for kk in range(KCH):
                    q.dma_start(w_sb[:, kk],
                                w_in[128 * kk:128 * (kk + 1), :])
                    q.dma_start(x_sb[:, kk, 0:SQG],
                                x_in[128 * kk:128 * (kk + 1), 0:SQG])
            # v for sg=0 is split by column half across BOTH rings: the
            # v-projection (and the first PV matmuls behind it) only needs
            # cols 0:256 of every kk chunk, so half 0 unblocks ~5us earlier
            # than a kk-major full-width load order would allow.
            for kk in range(KCH):
                nc.sync.dma_start(wv_sb[:, kk],
                                  wv[128 * kk:128 * (kk + 1), :])
                nc.sync.dma_start(xv_sb[:, kk, 0:256],
                                  xv[128 * kk:128 * (kk + 1), 0:256])
            for kk in range(KCH):
                nc.scalar.dma_start(xv_sb[:, kk, 256:SQG],
                                    xv[128 * kk:128 * (kk + 1), 256:SQG])
            for sg in range(1, NSQG):
                for x_in, x_sb, q in ((xq, xq_sb, nc.sync),
                                      (xk, xk_sb, nc.scalar),
                                      (xv, xv_sb, nc.sync)):
                    for kk in range(KCH):
                        q.dma_start(
                            x_sb[:, kk, SQG * sg:SQG * (sg + 1)],
                            x_in[128 * kk:128 * (kk + 1),
                                 SQG * sg:SQG * (sg + 1)])
                if sg == 1:
                    for kc in range(2):
                        nc.sync.dma_start(wo_sb[:, kc],
                                          wo[128 * kc:128 * (kc + 1), :])
            # small constants go on the gpsimd queue, off the critical path
            mask_sb = wpool.tile([128, 128], DT, tag="mask")
            nc.gpsimd.dma_start(mask_sb[:], mask_t[:])
            ones64 = wpool.tile([1, 64], DT, tag="ones64")
            nc.gpsimd.dma_start(ones64[:], ones_r[:])
            idn128 = wpool.tile([128, 128], DT, tag="idn128")
            nc.gpsimd.dma_start(idn128[:], idn128_p[:])
            sel_sb = wpool.tile([8, 8, 64], DT, tag="sel")
            nc.gpsimd.dma_start(sel_sb[:], sel_p[:])
            wo2_sb = wpool.tile([128, KCH, DL], DT, tag="wo2")
            for kk in range(KCH):
                nc.gpsimd.dma_start(wo2_sb[:, kk],
                                    wo2[128 * kk:128 * (kk + 1), :])

            # warm the exp table set early (overlaps with first proj)
            warm2 = wpool.tile([1, 16], DT, tag="warm2")
            nc.scalar.activation(warm2[:], ones64[:, :16], AF.Exp, scale=1.0)

            # ---- persistent activations ----
            qT = [apool.tile([128, S], DT, tag=f"qT{hp}", name=f"qT{hp}")
                  for hp in range(2)]
            kT = [apool.tile([128, S], DT, tag=f"kT{hp}", name=f"kT{hp}")
                  for hp in range(2)]
            Vp = apool.tile([128, NSK, 65 * HL], DT, tag="Vp")
            nc.gpsimd.dma_start(
                Vp.rearrange("p i (h e) -> p i h e", e=65)[:, :, :, 64:65],
                ones_c.rearrange("p (i h one) -> p i h one", h=HL, one=1))
            ctx_sb = apool.tile([128, 2, S], DT, tag="ctx")

            # -------- deferred-PE-task machinery --------
            # queue of closures, each emitting ~0.5-1us of PE work; drained
            # between attention i-blocks to keep the PE FIFO stall-free.
            work_q = []

            def drain(n):
                for _ in range(n):
                    if not work_q:
                        return
                    work_q.pop(0)()

            def drain_all():
                drain(len(work_q))

            # -------- projections for one sq group --------
            def queue_proj(sg):
                # q,k: feature-on-partition outputs qT/kT [(2x128), 512]
                for tname, x_sb, w_sb, dst in (
                        ("q", xq_sb, wq_sb, qT), ("k", xk_sb, wk_sb, kT)):
                    def emit_qk(tname=tname, x_sb=x_sb, w_sb=w_sb,
                                dst=dst, sg=sg):
                        pss = [psP.tile([128, SQG], dt.float32, tag=f"pp{cc}",
                                        name=f"pj{tname}{cc}", bufs=1)
                               for cc in range(2)]
                        for kk in range(KCH):
                            for cc in range(2):
                                nc.tensor.matmul(
                                    pss[cc][:],
                                    lhsT=w_sb[:, kk, 128 * cc:128 * (cc + 1)],
                                    rhs=x_sb[:, kk,
                                             SQG * sg:SQG * (sg + 1)],
                                    start=(kk == 0), stop=(kk == KCH - 1))
                        for cc in range(2):
                            nc.vector.tensor_copy(
                                dst[cc][:, SQG * sg:SQG * (sg + 1)], pss[cc][:])
                    work_q.append(emit_qk)
                # v natural layout with the ones column -> Vp
                def _v_half(half, sg=sg):
                    psv = [psP.tile([128, SQG], dt.float32, tag=f"pp{j}",
                                    name=f"pv{j}", bufs=1)
                           for j in range(2)]
                    for kk in range(KCH):
                        for j in range(2):
                            sc = 2 * half + j
                            nc.tensor.matmul(
                                psv[j][:, :DL],
                                lhsT=xv_sb[:, kk,
                                           SQG * sg + 128 * sc:
                                           SQG * sg + 128 * (sc + 1)],
                                rhs=wv_sb[:, kk, :],
                                start=(kk == 0), stop=(kk == KCH - 1))
                    for j in range(2):
                        sc = 2 * half + j
                        i = 4 * sg + sc
                        vdst = Vp[:, i].rearrange("p (h e) -> p h e", e=65)
                        nc.vector.tensor_copy(
                            vdst[:, :, :64],
                            psv[j][:, :DL]
                            .rearrange("p (h e) -> p h e", e=64))

                if sg == 0:
                    # split so Vp chunks 0-1 (all the first PV matmuls need)
                    # unblock as soon as the xv col-half-0 DMAs land
                    work_q.append(lambda: _v_half(0))
                    work_q.append(lambda: _v_half(1))
                else:
                    def emit_v():
                        _v_half(0)
                        _v_half(1)
                    work_q.append(emit_v)

            # ---- output projection + ReduceScatter for one sq window ----
            out_dmas = []   # deferred (rsout, w0, w1): emitted at kernel end

            def queue_outproj_w(idx, w0, w1):
                F = w1 - w0
                par_sb = opool.tile([128, KCH, F], DT, tag="par",
                                    name=f"par{idx}", bufs=2)
                # unique tags: staging buffers must NOT share a DRAM slot,
                # else part(k+1) DMAs wait on RS(k)
                part = drp.tile([KCH, 128, F], dt.float16,
                                name=f"part{idx}", tag=f"part{idx}")

                def emit_oc(oc, par_sb=par_sb, part=part, w0=w0, w1=w1):
                    def f():
                        pso = psP.tile([128, F], dt.float32,
                                       tag=f"pp{oc % 2}", name=f"po{oc % 2}",
                                       bufs=1)
                        for kc in range(2):
                            nc.tensor.matmul(
                                pso[:],
                                lhsT=wo_sb[:, kc, 128 * oc:128 * (oc + 1)],
                                rhs=ctx_sb[:, kc, w0:w1],
                                start=(kc == 0), stop=(kc == 1))
                        # alternate DVE/ScalarE so the 8 evacuation copies
                        # pipeline on two engines (shortens the RS trigger
                        # path after the last attention block)
                        if oc % 2 == 0:
                            nc.vector.tensor_copy(par_sb[:, oc, :], pso[:])
                        else:
                            nc.scalar.copy(par_sb[:, oc, :], pso[:])
                        # gpsimd queue: keeps the RS staging off the Sync
                        # prefetch stream (and vice versa)
                        nc.gpsimd.dma_start(part[oc], par_sb[:, oc, :])
                    return f
                for oc in range(KCH):
                    work_q.append(emit_oc(oc))

                def emit_rs(part=part, idx=idx, w0=w0, w1=w1):
                    rsout = drp.tile([2, 128, F], dt.float16,
                                     name=f"rso{idx}", tag=f"rso{idx}")
                    nc.gpsimd.collective_compute(
                        "ReduceScatter", ALU.add, replica_groups=GROUPS,
                        ins=[part.opt()], outs=[rsout.opt()])
                    # the out DMA waits on the RS; emitting it here would
                    # block the gpsimd queue (and the next RS's staging)
                    # behind this RS's completion -- defer to the end, on
                    # the by-then-idle Sync queue.
                    out_dmas.append((rsout, w0, w1))
                work_q.append(emit_rs)

            def queue_outproj_ag(idx, w0, w1):
                # tail-only path: AllGather this core's ctx slice, then a
                # fully local out-projection for its 256 output features.
                F = w1 - w0
                ctxd = drp.tile([2, 128, F], dt.float16,
                                name=f"ctxd{idx}", tag=f"ctxd{idx}")
                ctxg = drp.tile([4, 2, 128, F], dt.float16,
                                name=f"ctxg{idx}", tag=f"ctxg{idx}")
                ctxg_sb = opool.tile([128, KCH, F], DT, tag="ctxg",
                                     name=f"ctxg_sb{idx}", bufs=1)

                def emit_ag():
                    for kc in range(2):
                        nc.gpsimd.dma_start(ctxd[kc], ctx_sb[:, kc, w0:w1])
                    nc.gpsimd.collective_compute(
                        "AllGather", ALU.bypass, replica_groups=GROUPS,
                        ins=[ctxd.opt()], outs=[ctxg.opt()])
                work_q.append(emit_ag)

                def emit_load():
                    for r in range(4):
                        for kc in range(2):
                            nc.gpsimd.dma_start(ctxg_sb[:, 2 * r + kc, :],
                                                ctxg[r, kc])
                work_q.append(emit_load)

                def emit_oproj(oc2):
                    def f():
                        pso = psP.tile([128, F], dt.float32,
                                       tag=f"pp{oc2}", name=f"po{oc2}",
                                       bufs=1)
                        for kk in range(KCH):
                            nc.tensor.matmul(
                                pso[:],
                                lhsT=wo2_sb[:, kk,
                                            128 * oc2:128 * (oc2 + 1)],
                                rhs=ctxg_sb[:, kk, :],
                                start=(kk == 0), stop=(kk == KCH - 1))
                        par2 = opool.tile([128, F], DT, tag="par2",
                                          name=f"par2_{oc2}", bufs=2)
                        if oc2 == 0:
                            nc.vector.tensor_copy(par2[:], pso[:])
                        else:
                            nc.scalar.copy(par2[:], pso[:])
                        nc.sync.dma_start(out[oc2, :, w0:w1], par2[:])
                    return f
                for oc2 in range(2):
                    work_q.append(emit_oproj(oc2))

            # -------- attention for one sq window [w0, w1) --------
            def attn_w(w0, w1):
                F = w1 - w0
                nW = F // 128
                for hp in range(2):
                    nsk = w1 // 128 if causal else NSK
                    # one 65-row ctx'+denom accumulator bank per packed head
                    ctx_ps = psC.tile([128, F], dt.float32, tag="ctx",
                                      name=f"ctx{w0}{hp}", bufs=1)
                    ctx_ps2 = psC.tile([128, F], dt.float32, tag="ctx2",
                                       name=f"ctx2{w0}{hp}", bufs=1)
                    ctxs = [ctx_ps, ctx_ps2]
                    pend_pv = []   # deferred PV emissions

                    def flush_pv(n):
                        for _ in range(min(n, len(pend_pv))):
                            pend_pv.pop(0)()

                    for i in range(nsk):
                        col0 = max(0, 128 * i - w0) if causal else 0
                        # always 2xSQG: each head's scores must land in its
                        # OWN psum bank -- with F=256 a [128,2,F] tile would
                        # put both heads in one bank and the two row-tiled
                        # concurrent matmuls would collide fatally
                        sps = psS.tile([128, 2, SQG], dt.float32, tag="sc",
                                       name=f"sps{w0}{hp}{i}")
                        for m in range(2):
                            nc.tensor.matmul(
                                sps[:, m, col0:F],
                                lhsT=kT[hp][64 * m:64 * m + 64,
                                            128 * i:128 * (i + 1)],
                                rhs=qT[hp][64 * m:64 * m + 64,
                                           w0 + col0:w1],
                                start=True, stop=True,
                                tile_position=(64 * m, 0))
                        et = epool.tile([128, 2, SQG], DT, tag="exp",
                                        name=f"et{w0}{hp}{i}")
                        nc.scalar.activation(
                            et[:, :, col0:F], sps[:, :, col0:F],
                            AF.Exp, scale=SCALE)
                        if causal and 128 * i >= w0:
                            for m in range(2):
                                nc.vector.tensor_tensor(
                                    et[:, m, col0:col0 + 128],
                                    et[:, m, col0:col0 + 128],
                                    mask_sb[:], ALU.mult)

                        def emit_pv(i=i, col0=col0, et=et, hp=hp, F=F,
                                    first=(i == 0), last=(i == nsk - 1)):
                            for m in range(2):
                                hl = 2 * hp + m
                                nc.tensor.matmul(
                                    ctxs[m][0:65, col0:F],
                                    lhsT=Vp[:, i, 65 * hl:65 * hl + 65],
                                    rhs=et[:, m, col0:F],
                                    start=first, stop=last)
                        pend_pv.append(emit_pv)
                        # keep PE busy: one deferred task + the PV from two
                        # blocks ago (its exp has long finished)
                        drain(1)
                        if len(pend_pv) > 2:
                            flush_pv(1)
                    flush_pv(len(pend_pv))

                    # normalize ctx rows by 1/denom.  A naive DVE reciprocal
                    # on [1,F] runs F elems on ONE lane at ~6 cyc/elem.
                    # Instead transpose the two denom rows onto 128
                    # partitions via tiny PE matmuls, take the reciprocal on
                    # [128,2nW] (a few elems/lane), transpose back, and
                    # broadcast with selector-outer-product matmuls.
                    d_sb = [opool.tile([1, SQG], DT, tag=f"d{m}",
                                       name=f"d{m}") for m in range(2)]
                    for m in range(2):
                        nc.vector.tensor_copy(d_sb[m][0:1, 0:F],
                                              ctxs[m][64:65, :])
                    dT = psP.tile([128, 4, 2], dt.float32, tag="pp0",
                                  name="dT", bufs=1)
                    for c in range(nW):
                        for m in range(2):
                            nc.tensor.matmul(
                                dT[:, c, m:m + 1],
                                lhsT=d_sb[m][0:1, 128 * c:128 * (c + 1)],
                                rhs=ones64[:, 0:1],
                                start=True, stop=True)
                    rT = opool.tile([128, 4, 2], dt.float32, tag="rT",
                                    name="rT")
                    nc.vector.reciprocal(rT[:, 0:nW, :], dT[:, 0:nW, :])
                    rT16 = opool.tile([128, 4, 2], DT, tag="rT16",
                                      name="rT16")
                    nc.vector.tensor_copy(rT16[:, 0:nW, :], rT[:, 0:nW, :])
                    rr = psP.tile([8, 128], dt.float32, tag="pp1",
                                  name="rr", bufs=1)
                    nc.tensor.matmul(
                        rr[0:2 * nW, :],
                        lhsT=rT16[:, 0:nW, :].rearrange("p c m -> p (c m)"),
                        rhs=idn128[:], start=True, stop=True)
                    rr_sb = opool.tile([8, 128], DT, tag="rr", name="rr_sb")
                    nc.vector.tensor_copy(rr_sb[0:2 * nW, :], rr[0:2 * nW, :])
                    for m in range(2):
                        bc = psP.tile([64, SQG], dt.float32, tag="pp0",
                                      name=f"bc{m}", bufs=1)
                        for c in range(nW):
                            # bc[:, chunk c] = (e_{2c+m} x ones64).T @ rr_sb
                            #               = ones64 outer rr row 2c+m
                            nc.tensor.matmul(
                                bc[:, 128 * c:128 * (c + 1)],
                                lhsT=sel_sb[0:2 * nW, 2 * c + m, :],
                                rhs=rr_sb[0:2 * nW, :],
                                start=True, stop=True)
                        bc_sb = opool.tile([64, SQG], dt.float32,
                                           tag="bcsb", name=f"bc_sb{m}")
                        nc.vector.tensor_copy(bc_sb[:, 0:F], bc[:, 0:F])
                        nc.vector.tensor_tensor(
                            ctx_sb[64 * m:64 * m + 64, hp, w0:w1],
                            ctxs[m][0:64, :],
                            bc_sb[:, 0:F], ALU.mult)

            # -------- top-level schedule --------
            # the last sq group is split into two 256-wide windows so its
            # first half's ReduceScatter overlaps the second half's
            # attention -- only a 512 KB RS remains exposed in the tail.
            if os.environ.get("KERNEL_WINDOWS", "4") == "4":
                WINDOWS = [(0, 512), (512, 1024), (1024, 1536), (1536, 2048)]
            else:
                WINDOWS = [(0, 512), (512, 1024), (1024, 1536),
                           (1536, 1792), (1792, 2048)]
            queue_proj(0)
            drain_all()
            for wi, (w0, w1) in enumerate(WINDOWS):
                sg = w0 // SQG
                if w0 % SQG == 0 and sg + 1 < NSQG:
                    queue_proj(sg + 1)
                attn_w(w0, w1)
                if wi == len(WINDOWS) - 1:
                    queue_outproj_ag(wi, w0, w1)
                    drain_all()
                else:
                    queue_outproj_w(wi, w0, w1)
            drain_all()
            for rsout, w0, w1 in out_dmas:
                nc.sync.dma_start(out[:, :, w0:w1], rsout[:])

    _split_multiwait(nc)
    return nc


def _mask_kind(mask: np.ndarray) -> bool:
    """True if causal (tril), False if all-ones; raises otherwise."""
    m = np.asarray(mask).reshape(S, S)
    if np.array_equal((m != 0).astype(np.int8),
                      np.tril(np.ones((S, S), np.int8))):
        return True
    if np.all(m != 0):
        return False
    raise NotImplementedError("unsupported mask pattern")


def kernel(q, k, v, mask, w_q, b_q, w_k, b_k, w_v, b_v, w_o, b_o):
    global LAST_RESULT
    assert not np.any(b_q) and not np.any(b_k) and not np.any(b_v) \
        and not np.any(b_o), "nonzero biases not supported"
    dtname = DTNAME
    npdt = _DT_NP[dtname]
    causal = _mask_kind(mask)

    key = (dtname, causal)
    if key not in _CACHE:
        _CACHE[key] = _build(dtname, causal)
    nc = _CACHE[key]

    q = np.asarray(q, np.float32)
    k = np.asarray(k, np.float32)
    v = np.asarray(v, np.float32)
    # transposed per-batch activations
    xqs = [np.ascontiguousarray(q[b].T).astype(npdt) for b in range(B)]
    xks = [np.ascontiguousarray(k[b].T).astype(npdt) for b in range(B)]
    xvs = [np.ascontiguousarray(v[b].T).astype(npdt) for b in range(B)]
    wqs = [np.ascontiguousarray(np.asarray(w_q, np.float32)[:, DL * g:DL * (g + 1)]).astype(npdt) for g in range(4)]
    wks = [np.ascontiguousarray(np.asarray(w_k, np.float32)[:, DL * g:DL * (g + 1)]).astype(npdt) for g in range(4)]
    wvs = [np.ascontiguousarray(np.asarray(w_v, np.float32)[:, DL * g:DL * (g + 1)]).astype(npdt) for g in range(4)]
    wos = [np.ascontiguousarray(np.asarray(w_o, np.float32)[DL * g:DL * (g + 1), :]).astype(npdt) for g in range(4)]
    wo2s = [np.ascontiguousarray(np.asarray(w_o, np.float32)[:, DL * g:DL * (g + 1)]).astype(npdt) for g in range(4)]
    onc = np.ones((128, 64), npdt)
    onr = np.ones((1, 64), npdt)
    # multiplicative tril tile in scores_T layout: valid iff p <= f
    mt = (np.arange(128)[:, None] <= np.arange(128)[None, :]).astype(npdt)

    in_maps = []
    for c in range(NCORE):
        b, g = c // 4, c % 4
        in_maps.append({
            "xq": xqs[b], "xk": xks[b], "xv": xvs[b],
            "wq": wqs[g], "wk": wks[g], "wv": wvs[g], "wo": wos[g],
            "wo2": wo2s[g],
            "mask_t": mt, "ones_c": onc, "ones_r": onr,
            "idn128": np.eye(128, dtype=npdt),
            "sel": np.ascontiguousarray(
                np.eye(8, dtype=npdt)[:, :, None]
                * np.ones((1, 1, 64), npdt)),
        })
    res = run_bass_kernel_spmd(nc, in_maps, core_ids=list(range(NCORE)))
    LAST_RESULT = res

    outf = np.empty((B, S, D), np.float32)
    for c in range(NCORE):
        b, g = c // 4, c % 4
        o = res.results[c]["out"].reshape(DL, S).astype(np.float32)
        outf[b, :, DL * g:DL * (g + 1)] = o.T
    return outf
